# revision 1
# baseline (speedup 1.0000x reference)
"""Radix-2 DIF variant: contraction halved by pre-combining lag-product halves.

X[k, 2t]   = sum_{m<512} (R[k,m]+R[k,m+512]) * w512^{mt}
X[k, 2t+1] = sum_{m<512} (R[k,m]-R[k,m+512]) * w^m * w512^{mt}

Rsum/Rdiff are built on VectorE from sliding-window tiles (negative free-step
reads are legal on DVE), so the matmul stationaries are plain ascending slices
and the output comes out with k ascending (no J-flip on the direct path).
The w^m twiddle and the f-axis fftshift are baked into constant DFT tables
(stationary-free moving operands, resident in SBUF).
"""

import numpy as np

import bass_rust
import concourse.bass as bass
import concourse.mybir as mybir
import concourse.tile as tile
import concourse.bass_utils as bass_utils

B, N = 16, 1024
NCORES = 8
BPC = B // NCORES
NKB = 5  # k-blocks: k in [0, 640)
DS_LEN = 2176

f32 = mybir.dt.float32
f32r = mybir.dt.float32r
ALU = mybir.AluOpType


def _split_excess_waits(nc):
    for f in nc.m.functions:
        for blk in f.blocks:
            insts = list(blk.instructions)
            new_insts = []
            changed = False
            for inst in insts:
                si = inst.sync_info
                waits = list(si.on_wait) if (si is not None and si.on_wait) else []
                keep_n = 0 if isinstance(inst, mybir.InstDrain) else 1
                if len(waits) > keep_n:
                    changed = True
                    extra = waits[: len(waits) - keep_n]
                    keep = waits[len(waits) - keep_n:]
                    for w in extra:
                        nop = mybir.InstNoOp(
                            name=nc.get_next_instruction_name(), ins=[], outs=[]
                        )
                        nop.engine = inst.engine
                        nop.sync_info = bass_rust.SyncInfo(on_wait=[w], on_update=[])
                        new_insts.append(nop)
                    inst.sync_info = bass_rust.SyncInfo(
                        on_wait=keep,
                        on_update=list(si.on_update) if si.on_update else [],
                    )
                new_insts.append(inst)
            if changed:
                blk.instructions = new_insts
    return nc


TABNAMES = ["tec", "tes", "tesn", "toc", "tos", "tosn"]


def build_nc():
    nc = bass.Bass("TRN2", target_bir_lowering=False, debug=False)

    dsr = nc.dram_tensor("dsr", [BPC, DS_LEN], f32r, kind="ExternalInput")
    dsi = nc.dram_tensor("dsi", [BPC, DS_LEN], f32r, kind="ExternalInput")
    dsni = nc.dram_tensor("dsni", [BPC, DS_LEN], f32r, kind="ExternalInput")
    scols = nc.dram_tensor("scols", [BPC, 128, 16], f32, kind="ExternalInput")
    tabs = {
        nm: nc.dram_tensor(nm, [512, 512], f32r, kind="ExternalInput")
        for nm in TABNAMES
    }
    jmat = nc.dram_tensor("jmat", [128, 128], f32r, kind="ExternalInput")
    out = nc.dram_tensor("out", [BPC, N, N], f32, kind="ExternalOutput")

    with tile.TileContext(nc) as tc:
        with (
            tc.tile_pool(name="const", bufs=1) as constp,
            tc.tile_pool(name="tp", bufs=1) as tp,
            tc.tile_pool(name="rp", bufs=1) as rp,
            tc.tile_pool(name="tmp", bufs=2) as tmpp,
            tc.tile_pool(name="u", bufs=1) as up,
            tc.tile_pool(name="chi", bufs=1) as chip,
            tc.tile_pool(name="st", bufs=2) as stp,
            tc.tile_pool(name="ms", bufs=1) as msp,
            tc.tile_pool(name="mj", bufs=2) as mjp,
            tc.tile_pool(name="sm", bufs=1) as smp,
            tc.tile_pool(name="ps", bufs=2, space="PSUM") as psp,
        ):
            tJ = constp.tile([128, 128], f32r, tag="jmat")
            nc.scalar.dma_start(tJ[:], jmat[:])
            # resident DFT tables, per 128-chunk of m
            TT = {}
            k = 0
            for q in range(4):  # q-major: chunk-0 tables land first
                for nm in TABNAMES:
                    t = constp.tile([128, 512], f32r, tag=f"{nm}{q}")
                    TT[(nm, q)] = t
            def load_tab(nm, q, eng):
                eng.dma_start(TT[(nm, q)][:], tabs[nm][q * 128:(q + 1) * 128, :])

            def emit_load(b):
                s = {"b": b, "chis": [], "R": [None] * 4}
                scol = smp.tile([128, 16], f32, tag=f"scol{b}")
                nc.sync.dma_start(scol[:], scols[b])
                s["scol"] = scol
                Tsr = tp.tile([128, 1536], f32r, tag="tsr")
                Tsi = tp.tile([128, 1536], f32r, tag="tsi")
                Tnsi = tp.tile([128, 1536], f32r, tag="tnsi")
                nc.sync.dma_start(Tsr[:], bass.AP(dsr, b * DS_LEN + 385, [[1, 128], [1, 1536]]))
                nc.scalar.dma_start(Tsi[:], bass.AP(dsi, b * DS_LEN + 385, [[1, 128], [1, 1536]]))
                nc.gpsimd.dma_start(Tnsi[:], bass.AP(dsni, b * DS_LEN + 385, [[1, 128], [1, 1536]]))
                s["T"] = (Tsr, Tsi, Tnsi)
                rowall = smp.tile([1, 640], f32, tag=f"rowall{b}")
                s["rowall"] = rowall
                return s

            def win(T, off):
                # [p, kk] -> T[p, off - kk], kk in [0, 640)
                ap = T[:]
                return bass.AP(ap.tensor, ap.offset + off, [ap.ap[0], [-1, 640]])

            def emit_rbuild(s, qs, lo=0, hi=640):
                # R^T[m, kk] = s[m] * conj(s)[(m-kk)%N]; sum/diff of m and m+512.
                # Built in k-column slices so early k-blocks unblock sooner.
                Tsr, Tsi, Tnsi = s["T"]
                scol = s["scol"]
                n = hi - lo
                for q in qs:
                    m0 = 128 * q
                    terms = []
                    for half, woff in ((0, 1024 + m0), (1, 1536 + m0)):
                        sr_c = scol[:, q + 4 * half:q + 4 * half + 1]
                        si_c = scol[:, 8 + q + 4 * half:9 + q + 4 * half]
                        def w(T):
                            ap = T[:]
                            return bass.AP(ap.tensor, ap.offset + woff - 385 - lo, [ap.ap[0], [-1, n]])
                        w_sr, w_si, w_nsi = w(Tsr), w(Tsi), w(Tnsi)
                        a = tmpp.tile([128, 640], f32, tag="ta")
                        ur = up.tile([128, 640], f32, tag=f"ur{half}")
                        # Rr = sr_m*csr + si_m*si_win
                        nc.vector.tensor_scalar_mul(a[:, 0:n], w_sr, sr_c)
                        nc.vector.scalar_tensor_tensor(
                            ur[:, 0:n], w_si, si_c, a[:, 0:n], op0=ALU.mult, op1=ALU.add
                        )
                        b2 = tmpp.tile([128, 640], f32, tag="tb")
                        ui = up.tile([128, 640], f32, tag=f"ui{half}")
                        # Ri = si_m*csr - sr_m*si_win  (= si_m*csr + sr_m*(-si_win))
                        nc.vector.tensor_scalar_mul(b2[:, 0:n], w_nsi, sr_c)
                        nc.vector.scalar_tensor_tensor(
                            ui[:, 0:n], w_sr, si_c, b2[:, 0:n], op0=ALU.mult, op1=ALU.add
                        )
                        terms.append((ur, ui))
                    (u1r, u1i), (u2r, u2i) = terms
                    if lo == 0:
                        qt = f"0_{s['b'] % 2}" if q == 0 else str(q)
                        rsr = rp.tile([128, 640], f32r, tag=f"rsr{qt}")
                        rsi = rp.tile([128, 640], f32r, tag=f"rsi{qt}")
                        rdr = rp.tile([128, 640], f32r, tag=f"rdr{qt}")
                        rdi = rp.tile([128, 640], f32r, tag=f"rdi{qt}")
                    else:
                        rsr, rsi, rdr, rdi = s["R"][q]
                    nc.vector.scalar_tensor_tensor(
                        rsr[:, lo:hi], u1r[:, 0:n], 1.0, u2r[:, 0:n], op0=ALU.mult, op1=ALU.add)
                    nc.vector.scalar_tensor_tensor(
                        rdr[:, lo:hi], u1r[:, 0:n], 1.0, u2r[:, 0:n], op0=ALU.mult, op1=ALU.subtract)
                    nc.vector.scalar_tensor_tensor(
                        rsi[:, lo:hi], u1i[:, 0:n], 1.0, u2i[:, 0:n], op0=ALU.mult, op1=ALU.add)
                    nc.vector.scalar_tensor_tensor(
                        rdi[:, lo:hi], u1i[:, 0:n], 1.0, u2i[:, 0:n], op0=ALU.mult, op1=ALU.subtract)
                    s["R"][q] = (rsr, rsi, rdr, rdi)

            def emit_kblock(b, s, kb):
                c = 128 * kb
                xre = psp.tile([128, 512], f32, tag="xre")
                xie = psp.tile([128, 512], f32, tag="xie")
                xro = psp.tile([128, 512], f32, tag="xro")
                xio = psp.tile([128, 512], f32, tag="xio")
                for q in range(4):
                    rsr, rsi, rdr, rdi = s["R"][q]
                    first = q == 0
                    last = q == 3
                    psr = rsr[:, c:c + 128]
                    psi = rsi[:, c:c + 128]
                    pdr = rdr[:, c:c + 128]
                    pdi = rdi[:, c:c + 128]
                    nc.tensor.matmul(xre[:], psr, TT[("tec", q)][:], start=first, stop=False)
                    nc.tensor.matmul(xie[:], psi, TT[("tec", q)][:], start=first, stop=False)
                    nc.tensor.matmul(xro[:], pdr, TT[("toc", q)][:], start=first, stop=False)
                    nc.tensor.matmul(xio[:], pdi, TT[("toc", q)][:], start=first, stop=False)
                    nc.tensor.matmul(xre[:], psi, TT[("tes", q)][:], start=False, stop=last)
                    nc.tensor.matmul(xie[:], psr, TT[("tesn", q)][:], start=False, stop=last)
                    nc.tensor.matmul(xro[:], pdi, TT[("tos", q)][:], start=False, stop=last)
                    nc.tensor.matmul(xio[:], pdr, TT[("tosn", q)][:], start=False, stop=last)

                chi_t = chip.tile([128, N], f32, tag=f"chi{(5 * b + kb) % 6}")
                tmax2 = smp.tile([128, 2], f32, tag=f"tmax{b}")
                for parity, (xr, xi) in ((0, (xre, xie)), (1, (xro, xio))):
                    sqa = tmpp.tile([128, 512], f32, tag="ta")
                    sqb = tmpp.tile([128, 512], f32, tag="tb")
                    nc.scalar.square(sqa[:], xr[:])
                    nc.scalar.square(sqb[:], xi[:])
                    cap = chi_t[:]
                    strided = bass.AP(cap.tensor, cap.offset + parity, [cap.ap[0], [2, 512]])
                    nc.vector.tensor_add(strided, sqa[:], sqb[:])
                    nc.vector.tensor_reduce(
                        tmax2[:, parity:parity + 1], strided,
                        axis=mybir.AxisListType.X, op=ALU.max,
                    )
                tmax1 = smp.tile([128, 1], f32, tag=f"tmax1_{b}")
                nc.vector.tensor_max(tmax1[:], tmax2[:, 0:1], tmax2[:, 1:2])
                # transpose this block's per-partition max into the row
                # accumulator now, so the final reduce is one short chain
                nc.sync.dma_start(s["rowall"][0:1, 128 * kb:128 * (kb + 1)], tmax1[:])
                s["chis"].append(chi_t)

            def emit_finalize(b, s):
                gmax = smp.tile([1, 1], f32, tag=f"gmax{b}")
                nc.vector.tensor_reduce(
                    gmax[:], s["rowall"][:], axis=mybir.AxisListType.X, op=ALU.max
                )
                bmax = smp.tile([128, 1], f32, tag=f"bmax{b}")
                nc.sync.dma_start(
                    bmax[:], bass.AP(gmax[:].tensor, gmax[:].offset, [[1, 1], [0, 128]])
                )
                binv = smp.tile([128, 1], f32, tag=f"binv{b}")
                nc.vector.reciprocal(binv[:], bmax[:])
                s["binv"] = binv

            def emit_direct(b, s, kbs):
                # k is already ascending: scale + store
                binv = s["binv"]
                for kb in kbs:
                    stg = stp.tile([128, N], f32, tag="stg")
                    nc.vector.tensor_scalar_mul(stg[:], s["chis"][kb][:], binv[:])
                    r0 = (128 * kb + 512) % N
                    eng = nc.sync if kb % 2 == 0 else nc.scalar
                    eng.dma_start(out[b, r0:r0 + 128, :], stg[:])

            def emit_mirror_flip(b, s, kbs):
                # f-reverse chi[k2] rows (k2 in [1,384] live in kb 0..3)
                s.setdefault("ms", {})
                for kb in kbs:
                    chi_t = s["chis"][kb]
                    ms = msp.tile([128, N], f32r, tag=f"ms{kb % 2}")
                    ap = chi_t[:]
                    rev = bass.AP(ap.tensor, ap.offset + 1023, [ap.ap[0], [-1, 1023]])
                    nc.vector.tensor_copy(ms[:, 0:1], chi_t[:, 0:1])
                    nc.vector.tensor_copy(ms[:, 1:1024], rev)
                    s["ms"][kb] = ms

            def emit_mirror_jcopy(b, s, kbs):
                # J-flip (k asc -> desc) + unscaled PSUM->SBUF copy; no binv
                # dependency, so this overlaps the remaining k-blocks
                s.setdefault("mj", {})
                for kb in kbs:
                    ms = s["ms"][kb]
                    mj = mjp.tile([128, N], f32, tag=f"mj{kb % 2}")
                    for h in range(2):
                        hs = 512 * h
                        jy = psp.tile([128, 512], f32, tag=("xre" if h == 0 else "xro"))
                        nc.tensor.matmul(jy[:], tJ[:], ms[:, hs:hs + 512], start=True, stop=True)
                        nc.scalar.copy(mj[:, hs:hs + 512], jy[:])
                    s["mj"][kb] = mj

            def emit_mirror_store(b, s, kbs):
                # scale in place once 1/max is known, then store:
                # source partition r holds k2 = c+127-r -> dest row 385-c+r
                binv = s["binv"]
                for kb in kbs:
                    c = 128 * kb
                    mj = s["mj"][kb]
                    nc.scalar.mul(mj[:], mj[:], binv[:])
                    eng = nc.scalar if kb % 2 == 0 else nc.sync
                    if kb == 0:
                        eng.dma_start(out[b, 385:512, :], mj[0:127, :])
                    elif kb == 3:
                        eng.dma_start(out[b, 128:129, :], mj[127:128, :])
                    else:
                        r0 = 385 - c
                        eng.dma_start(out[b, r0:r0 + 128, :], mj[:])

            # --- pipelined schedule
            s0 = emit_load(0)
            for nm in TABNAMES:
                load_tab(nm, 0, nc.sync if nm in ("tec", "tes", "tesn") else nc.scalar)
            emit_rbuild(s0, [0])
            for q in (1, 2, 3):
                for i, nm in enumerate(TABNAMES):
                    load_tab(nm, q, (nc.sync, nc.scalar, nc.gpsimd)[i % 3])
            emit_rbuild(s0, [1, 2, 3], 0, 320)
            emit_rbuild(s0, [1, 2, 3], 320, 640)
            for kb in range(4):
                emit_kblock(0, s0, kb)
            s1 = emit_load(1)
            emit_rbuild(s1, [0])
            emit_kblock(0, s0, 4)
            emit_finalize(0, s0)
            emit_rbuild(s1, [1, 2, 3], 0, 320)
            emit_rbuild(s1, [1, 2, 3], 320, 640)
            emit_mirror_flip(0, s0, [0, 1])
            emit_mirror_jcopy(0, s0, [0, 1])
            emit_kblock(1, s1, 0)
            emit_kblock(1, s1, 1)
            emit_direct(0, s0, [0, 1])
            emit_mirror_store(0, s0, [0, 1])
            emit_kblock(1, s1, 2)
            emit_mirror_flip(0, s0, [2, 3])
            emit_mirror_jcopy(0, s0, [2, 3])
            emit_direct(0, s0, [2, 3])
            emit_mirror_store(0, s0, [2, 3])
            emit_kblock(1, s1, 3)
            emit_direct(0, s0, [4])
            emit_mirror_flip(1, s1, [0, 1])
            emit_mirror_jcopy(1, s1, [0, 1])
            emit_mirror_flip(1, s1, [2, 3])
            emit_mirror_jcopy(1, s1, [2, 3])
            emit_kblock(1, s1, 4)
            emit_finalize(1, s1)
            emit_direct(1, s1, [0, 1, 2, 3, 4])
            emit_mirror_store(1, s1, [0, 1, 2, 3])

    _split_excess_waits(nc)
    return nc


_NC_CACHE = {}


def _get_nc():
    if "nc" not in _NC_CACHE:
        _NC_CACHE["nc"] = build_nc()
    return _NC_CACHE["nc"]


def _get_tables():
    if "tabs" not in _NC_CACHE:
        m = np.arange(512, dtype=np.float64)[:, None]
        tp_ = np.arange(512, dtype=np.float64)[None, :]
        t_of = (tp_ + 256) % 512
        ang_e = 2.0 * np.pi * ((m * t_of) % 512) / 512
        ang_o = ang_e + 2.0 * np.pi * m / 1024
        tabs = {
            "tec": np.cos(ang_e).astype(np.float32),
            "tes": np.sin(ang_e).astype(np.float32),
            "toc": np.cos(ang_o).astype(np.float32),
            "tos": np.sin(ang_o).astype(np.float32),
        }
        tabs["tesn"] = -tabs["tes"]
        tabs["tosn"] = -tabs["tos"]
        _NC_CACHE["tabs"] = (tabs, np.eye(128, dtype=np.float32)[::-1].copy())
    return _NC_CACHE["tabs"]


def kernel(s_real: np.ndarray, s_imag: np.ndarray) -> np.ndarray:
    s_real = np.asarray(s_real, dtype=np.float32)
    s_imag = np.asarray(s_imag, dtype=np.float32)
    tabs, jnp_ = _get_tables()
    nc = _get_nc()

    in_maps = []
    for core in range(NCORES):
        sl = slice(core * BPC, (core + 1) * BPC)
        sr = s_real[sl]
        si = s_imag[sl]
        dsr = np.tile(sr, (1, 3))[:, :DS_LEN].copy()
        dsi_ = np.tile(si, (1, 3))[:, :DS_LEN].copy()
        scols = np.concatenate(
            [
                sr.reshape(BPC, 8, 128).transpose(0, 2, 1),
                si.reshape(BPC, 8, 128).transpose(0, 2, 1),
            ],
            axis=2,
        ).astype(np.float32).copy()
        im = {"dsr": dsr, "dsi": dsi_, "dsni": -dsi_, "scols": scols, "jmat": jnp_}
        im.update(tabs)
        in_maps.append(im)

    res = bass_utils.run_bass_kernel_spmd(nc, in_maps, core_ids=list(range(NCORES)))
    return np.concatenate([r["out"] for r in res.results], axis=0)



# revision 7
# speedup vs baseline: 1.2621x; 1.2621x over previous
"""Radix-4 DIF ambiguity kernel, bf16 lag products, host-folded normalization.

X[k, 4t+j] = sum_{m<256} B_j[m, k] * (w1024^{jm} w256^{mt})   (tables, bf16)
B_0 =  (R0+R2) + (R1+R3)     B_2 = (R0+R2) - (R1+R3)
B_1 =  (R0-R2) - i(R1-R3)    B_3 = (R0-R2) + i(R1-R3)     (Rl = R[m+256l])

The ambiguity max is always at the origin (Cauchy-Schwarz), so the max-
normalization is folded into a host-side input scaling s -> s/sqrt(sum|s|^2)
and the on-device reduce/normalize chain disappears. Only k in [0,512] is
computed (4.06 of 8 k-blocks); rows 1..511 are produced by the chi(-k,-f)
symmetry: f-reversal on DVE, k-reversal via identity-flip matmul (J), then
a plain PSUM->SBUF copy and store.
"""

import numpy as np
import ml_dtypes

import bass_rust
import concourse.bass as bass
import concourse.mybir as mybir
import concourse.tile as tile
import concourse.bass_utils as bass_utils

B, N = 16, 1024
NCORES = 8
BPC = B // NCORES
KW = 520  # k-width computed: kb0-3 full, kb4 holds k=512 (+7 pad)

f32 = mybir.dt.float32
f32r = mybir.dt.float32r
bf16 = mybir.dt.bfloat16
ALU = mybir.AluOpType

bf16np = ml_dtypes.bfloat16


def _split_excess_waits(nc):
    for f in nc.m.functions:
        for blk in f.blocks:
            insts = list(blk.instructions)
            new_insts = []
            changed = False
            for inst in insts:
                si = inst.sync_info
                waits = list(si.on_wait) if (si is not None and si.on_wait) else []
                keep_n = 0 if isinstance(inst, mybir.InstDrain) else 1
                if len(waits) > keep_n:
                    changed = True
                    extra = waits[: len(waits) - keep_n]
                    keep = waits[len(waits) - keep_n:]
                    for w in extra:
                        nop = mybir.InstNoOp(
                            name=nc.get_next_instruction_name(), ins=[], outs=[]
                        )
                        nop.engine = inst.engine
                        nop.sync_info = bass_rust.SyncInfo(on_wait=[w], on_update=[])
                        new_insts.append(nop)
                    inst.sync_info = bass_rust.SyncInfo(
                        on_wait=keep,
                        on_update=list(si.on_update) if si.on_update else [],
                    )
                new_insts.append(inst)
            if changed:
                blk.instructions = new_insts
    return nc


def build_nc():
    nc = bass.Bass("TRN2", target_bir_lowering=False, debug=False)

    dsr = nc.dram_tensor("dsr", [BPC, 2048], bf16, kind="ExternalInput")
    dS = nc.dram_tensor("dS", [BPC, 2048], bf16, kind="ExternalInput")
    dD = nc.dram_tensor("dD", [BPC, 2048], bf16, kind="ExternalInput")
    scols = nc.dram_tensor("scols", [BPC, 128, 24], f32, kind="ExternalInput")
    tabs = {}
    for j in range(4):
        for kind in ("c", "s", "sn"):
            nm = f"t{j}{kind}"
            tabs[(j, kind)] = nc.dram_tensor(nm, [256, 256], bf16, kind="ExternalInput")
    jmat = nc.dram_tensor("jmat", [128, 128], f32r, kind="ExternalInput")
    out = nc.dram_tensor("out", [BPC, N, N], f32, kind="ExternalOutput")

    with tile.TileContext(nc) as tc:
        with (
            tc.tile_pool(name="const", bufs=1) as constp,
            tc.tile_pool(name="win", bufs=2) as winp,
            tc.tile_pool(name="sm", bufs=2) as smp,
            tc.tile_pool(name="u", bufs=1) as up,
            tc.tile_pool(name="tmp", bufs=2) as tmpp,
            tc.tile_pool(name="r1", bufs=1) as r1p,
            tc.tile_pool(name="b2", bufs=2) as b2p,
            tc.tile_pool(name="sq", bufs=2) as sqp,
            tc.tile_pool(name="chi", bufs=4) as chip,
            tc.tile_pool(name="rev", bufs=1) as revp,
            tc.tile_pool(name="mj", bufs=1) as mjp,
            tc.tile_pool(name="ps", bufs=2, space="PSUM") as psp,
        ):
            tJ = constp.tile([128, 128], f32r, tag="jmat")
            nc.scalar.dma_start(tJ[:], jmat[:])
            TT = {}
            for j in range(4):
                for kind in ("c", "s", "sn"):
                    for c in range(2):
                        TT[(j, kind, c)] = constp.tile(
                            [128, 256], bf16, tag=f"t{j}{kind}{c}",
                            name=f"tt{j}{kind}{c}",
                        )

            def load_tab(j, kind, c, eng):
                eng.dma_start(
                    TT[(j, kind, c)][:], tabs[(j, kind)][128 * c:128 * (c + 1), :]
                )

            def emit_load(b):
                s = {"b": b, "chi": {}, "u": {}, "Rs": {}, "Rd": {}, "B": {}}
                scol = smp.tile([128, 24], f32, tag="scol")
                nc.sync.dma_start(scol[:], scols[b])
                s["scol"] = scol
                Tsr = winp.tile([128, 1921], bf16, tag="tsr")
                TS = winp.tile([128, 1921], bf16, tag="tS")
                TD = winp.tile([128, 1921], bf16, tag="tD")
                # row p = s_tiled[p : p+1921] (shift-by-1 rows via DRAM stride)
                nc.sync.dma_start(Tsr[:], bass.AP(dsr, b * 2048, [[1, 128], [1, 1921]]))
                nc.scalar.dma_start(TS[:], bass.AP(dS, b * 2048, [[1, 128], [1, 1921]]))
                nc.gpsimd.dma_start(TD[:], bass.AP(dD, b * 2048, [[1, 128], [1, 1921]]))
                s["T"] = (Tsr, TS, TD)
                return s

            def win(T, q8):
                # [p, kk] -> s_tiled[1024 + 128*q8 + p - kk], kk in [0, KW)
                # T row p = s_tiled[p + j]; read offset j = 1024 + 128*q8 - kk
                ap = T[:]
                return bass.AP(
                    ap.tensor, ap.offset + 1024 + 128 * q8, [ap.ap[0], [-1, KW]]
                )

            def emit_product(s, q8):
                # u[m,kk] = s[m] * conj(s)[(m-kk)%N]  (Gauss 3-mult)
                Tsr, TS, TD = s["T"]
                scol = s["scol"]
                sr_c = scol[:, q8:q8 + 1]
                si_c = scol[:, 8 + q8:9 + q8]
                ss_c = scol[:, 16 + q8:17 + q8]
                t = tmpp.tile([128, KW], bf16, tag="t")
                m2 = tmpp.tile([128, KW], bf16, tag="m2")
                m3 = tmpp.tile([128, KW], bf16, tag="m3")
                nc.vector.tensor_scalar_mul(t[:], win(Tsr, q8), ss_c)
                nc.vector.tensor_scalar_mul(m2[:], win(TD, q8), si_c)
                nc.vector.tensor_scalar_mul(m3[:], win(TS, q8), sr_c)
                ur = up.tile([128, KW], bf16, tag=f"ur{q8}")
                ui = up.tile([128, KW], bf16, tag=f"ui{q8}")
                nc.vector.tensor_add(ur[:], t[:], m2[:])
                nc.vector.tensor_sub(ui[:], t[:], m3[:])
                s["u"][q8] = (ur, ui)

            def emit_L1(s, c):
                # Rsum/Rdiff over m vs m+512: pairs (c, c+4), c in 0..3
                u1r, u1i = s["u"][c]
                u2r, u2i = s["u"][c + 4]
                rsr = r1p.tile([128, KW], bf16, tag=f"rsr{c}")
                rsi = r1p.tile([128, KW], bf16, tag=f"rsi{c}")
                rdr = r1p.tile([128, KW], bf16, tag=f"rdr{c}")
                rdi = r1p.tile([128, KW], bf16, tag=f"rdi{c}")
                nc.gpsimd.tensor_add(rsr[:], u1r[:], u2r[:])
                nc.gpsimd.tensor_add(rsi[:], u1i[:], u2i[:])
                nc.gpsimd.tensor_sub(rdr[:], u1r[:], u2r[:])
                nc.gpsimd.tensor_sub(rdi[:], u1i[:], u2i[:])
                s["Rs"][c] = (rsr, rsi)
                s["Rd"][c] = (rdr, rdi)

            def emit_L2(s, c):
                # B_j over m'' vs m''+256: pairs (c, c+2), c in 0..1
                asr, asi = s["Rs"][c]
                bsr, bsi = s["Rs"][c + 2]
                cdr, cdi = s["Rd"][c]
                ddr, ddi = s["Rd"][c + 2]
                for j, (x, y, op0, op1) in {
                    0: ((asr, asi), (bsr, bsi), ALU.add, ALU.add),
                    2: ((asr, asi), (bsr, bsi), ALU.subtract, ALU.subtract),
                    # B1 = c - i d: re = cr + di ; im = ci - dr
                    1: ((cdr, cdi), (ddi, ddr), ALU.add, ALU.subtract),
                    # B3 = c + i d: re = cr - di ; im = ci + dr
                    3: ((cdr, cdi), (ddi, ddr), ALU.subtract, ALU.add),
                }.items():
                    bre = b2p.tile([128, KW], bf16, tag=f"b{j}re{c}")
                    bim = b2p.tile([128, KW], bf16, tag=f"b{j}im{c}")
                    nc.vector.tensor_tensor(bre[:], x[0][:], y[0][:], op=op0)
                    nc.vector.tensor_tensor(bim[:], x[1][:], y[1][:], op=op1)
                    s["B"][(j, c)] = (bre, bim)

            def emit_kblock(s, kb):
                c0 = 128 * kb
                kwid = 128 if kb < 4 else 8
                pslc = slice(0, kwid)
                xt = {}
                for h in range(2):
                    xt[("re", h)] = psp.tile(
                        [128, 512], f32, tag=f"xre{h}", name=f"xre{h}_{kb}"
                    )
                    xt[("im", h)] = psp.tile(
                        [128, 512], f32, tag=f"xim{h}", name=f"xim{h}_{kb}"
                    )
                for j in range(4):
                    h, o = j // 2, 256 * (j % 2)
                    xre = xt[("re", h)][pslc, o:o + 256]
                    xim = xt[("im", h)][pslc, o:o + 256]
                    for c in range(2):
                        bre, bim = s["B"][(j, c)]
                        first, last = c == 0, c == 1
                        psr = bre[:, c0:c0 + kwid]
                        psi = bim[:, c0:c0 + kwid]
                        nc.tensor.matmul(xre, psr, TT[(j, "c", c)][:], start=first, stop=False)
                        nc.tensor.matmul(xim, psi, TT[(j, "c", c)][:], start=first, stop=False)
                        nc.tensor.matmul(xre, psi, TT[(j, "s", c)][:], start=False, stop=last)
                        nc.tensor.matmul(xim, psr, TT[(j, "sn", c)][:], start=False, stop=last)
                chi_t = chip.tile([128, N], f32, tag="chi")
                for h in range(2):
                    sqr = sqp.tile([128, 512], f32, tag=f"sqr{h}")
                    sqi = sqp.tile([128, 512], f32, tag=f"sqi{h}")
                    nc.scalar.square(sqr[pslc, :], xt[("re", h)][pslc, :])
                    nc.scalar.square(sqi[pslc, :], xt[("im", h)][pslc, :])
                    for jh in range(2):
                        j = 2 * h + jh
                        o = 256 * jh
                        cap = chi_t[pslc, :]
                        strided = bass.AP(
                            cap.tensor, cap.offset + j, [cap.ap[0], [4, 256]]
                        )
                        nc.vector.tensor_add(
                            strided, sqr[pslc, o:o + 256], sqi[pslc, o:o + 256]
                        )
                s["chi"][kb] = chi_t

            def emit_store_direct(s, kb, eng):
                b = s["b"]
                chi_t = s["chi"][kb]
                if kb < 4:
                    eng.dma_start(out[b, 512 + 128 * kb:640 + 128 * kb, :], chi_t[:])
                else:
                    eng.dma_start(out[b, 0:1, :], chi_t[0:1, :])

            def emit_mirror(s, kb):
                # chi_rev = [chi[0], chi[1023..1]] then J-flip rows via PE
                chi_t = s["chi"][kb]
                cr = revp.tile([128, N], f32r, tag=f"rev{kb % 2}")
                cap = chi_t[:]
                nc.vector.tensor_copy(cr[:, 0:1], chi_t[:, 0:1])
                rsrc = bass.AP(cap.tensor, cap.offset + 1023, [cap.ap[0], [-1, 1023]])
                nc.vector.tensor_copy(cr[:, 1:1024], rsrc)
                mj = mjp.tile([128, N], f32, tag=f"mj{kb % 2}")
                for h in range(2):
                    jy = psp.tile([128, 512], f32, tag=f"xre{h}")
                    nc.tensor.matmul(
                        jy[:], tJ[:], cr[:, 512 * h:512 * h + 512], start=True, stop=True
                    )
                    nc.scalar.copy(mj[:, 512 * h:512 * h + 512], jy[:])
                s["mj"] = s.get("mj", {})
                s["mj"][kb] = mj

            def emit_store_mirror(s, kb, eng):
                # mj partition p holds k1 = 128*kb + 127 - p -> out row 385-128*kb+p
                b = s["b"]
                mj = s["mj"][kb]
                if kb == 0:
                    eng.dma_start(out[b, 385:512, :], mj[0:127, :])
                else:
                    r0 = 385 - 128 * kb
                    eng.dma_start(out[b, r0:r0 + 128, :], mj[:])

            # ---- schedule
            s0 = emit_load(0)
            for j in range(4):
                load_tab(j, "c", 0, (nc.sync, nc.scalar, nc.gpsimd, nc.sync)[j])
            for q8 in (0, 4):
                emit_product(s0, q8)
            emit_L1(s0, 0)
            for j in range(4):
                load_tab(j, "s", 0, (nc.scalar, nc.gpsimd, nc.sync, nc.scalar)[j])
                load_tab(j, "sn", 0, (nc.gpsimd, nc.sync, nc.scalar, nc.gpsimd)[j])
            for q8 in (1, 5):
                emit_product(s0, q8)
            emit_L1(s0, 1)
            for j in range(4):
                for kind in ("c", "s", "sn"):
                    load_tab(j, kind, 1, (nc.sync, nc.scalar, nc.gpsimd)[j % 3])
            for q8 in (2, 6):
                emit_product(s0, q8)
            emit_L1(s0, 2)
            for q8 in (3, 7):
                emit_product(s0, q8)
            emit_L1(s0, 3)
            emit_L2(s0, 0)
            emit_L2(s0, 1)

            emit_kblock(s0, 0)
            emit_kblock(s0, 1)
            s1 = emit_load(1)
            emit_mirror(s0, 0)
            emit_store_direct(s0, 0, nc.sync)
            emit_store_mirror(s0, 0, nc.scalar)
            emit_kblock(s0, 2)
            for q8 in (0, 4):
                emit_product(s1, q8)
            emit_L1(s1, 0)
            emit_mirror(s0, 1)
            emit_store_direct(s0, 1, nc.scalar)
            emit_store_mirror(s0, 1, nc.sync)
            emit_kblock(s0, 3)
            for q8 in (1, 5):
                emit_product(s1, q8)
            emit_L1(s1, 1)
            emit_mirror(s0, 2)
            emit_store_direct(s0, 2, nc.sync)
            emit_store_mirror(s0, 2, nc.scalar)
            emit_kblock(s0, 4)
            for q8 in (2, 6):
                emit_product(s1, q8)
            emit_L1(s1, 2)
            emit_mirror(s0, 3)
            emit_store_direct(s0, 3, nc.scalar)
            emit_store_mirror(s0, 3, nc.sync)
            emit_store_direct(s0, 4, nc.sync)
            for q8 in (3, 7):
                emit_product(s1, q8)
            emit_L1(s1, 3)
            emit_L2(s1, 0)
            emit_L2(s1, 1)
            emit_kblock(s1, 0)
            emit_mirror(s1, 0)
            emit_store_direct(s1, 0, nc.sync)
            emit_store_mirror(s1, 0, nc.scalar)
            emit_kblock(s1, 1)
            emit_mirror(s1, 1)
            emit_store_direct(s1, 1, nc.scalar)
            emit_store_mirror(s1, 1, nc.sync)
            emit_kblock(s1, 2)
            emit_mirror(s1, 2)
            emit_store_direct(s1, 2, nc.sync)
            emit_store_mirror(s1, 2, nc.scalar)
            emit_kblock(s1, 3)
            emit_mirror(s1, 3)
            emit_store_direct(s1, 3, nc.scalar)
            emit_store_mirror(s1, 3, nc.sync)
            emit_kblock(s1, 4)
            emit_store_direct(s1, 4, nc.sync)

    _split_excess_waits(nc)
    return nc


_NC_CACHE = {}


def _get_nc():
    if "nc" not in _NC_CACHE:
        _NC_CACHE["nc"] = build_nc()
    return _NC_CACHE["nc"]


def _get_tables():
    if "tabs" not in _NC_CACHE:
        m = np.arange(256, dtype=np.float64)[:, None]
        tc_ = np.arange(256, dtype=np.float64)[None, :]
        t_of = (tc_ + 128) % 256
        tabs = {}
        for j in range(4):
            ang = 2.0 * np.pi * (m * j / 1024.0 + (m * t_of) % 256 / 256.0)
            tabs[f"t{j}c"] = np.cos(ang).astype(bf16np)
            tabs[f"t{j}s"] = np.sin(ang).astype(bf16np)
            tabs[f"t{j}sn"] = (-np.sin(ang)).astype(bf16np)
        _NC_CACHE["tabs"] = (tabs, np.eye(128, dtype=np.float32)[::-1].copy())
    return _NC_CACHE["tabs"]


def _make_in_maps(s_real, s_imag):
    s_real = np.asarray(s_real, dtype=np.float32)
    s_imag = np.asarray(s_imag, dtype=np.float32)
    tabs, jnp_ = _get_tables()
    E = (
        s_real.astype(np.float64) ** 2 + s_imag.astype(np.float64) ** 2
    ).sum(axis=1, keepdims=True)
    scale = E ** -0.5
    srn = (s_real * scale).astype(np.float32)
    sin_ = (s_imag * scale).astype(np.float32)

    in_maps = []
    for core in range(NCORES):
        sl = slice(core * BPC, (core + 1) * BPC)
        sr = srn[sl]
        si = sin_[sl]
        dsr = np.tile(sr, (1, 2)).astype(bf16np)
        dS = np.tile(sr + si, (1, 2)).astype(bf16np)
        dD = np.tile(si - sr, (1, 2)).astype(bf16np)
        scols = np.concatenate(
            [
                sr.reshape(BPC, 8, 128).transpose(0, 2, 1),
                si.reshape(BPC, 8, 128).transpose(0, 2, 1),
                (sr + si).reshape(BPC, 8, 128).transpose(0, 2, 1),
            ],
            axis=2,
        ).astype(np.float32).copy()
        im = {"dsr": dsr, "dS": dS, "dD": dD, "scols": scols, "jmat": jnp_}
        im.update(tabs)
        in_maps.append(im)
    return in_maps


def kernel(s_real: np.ndarray, s_imag: np.ndarray) -> np.ndarray:
    nc = _get_nc()
    in_maps = _make_in_maps(s_real, s_imag)
    res = bass_utils.run_bass_kernel_spmd(nc, in_maps, core_ids=list(range(NCORES)))
    return np.concatenate([r["out"] for r in res.results], axis=0)
